# revision 24
# baseline (speedup 1.0000x reference)
"""Multi-head causal attention on 8 Trainium2 NeuronCores.

Sharding: tensor-parallel over heads x data-parallel over batch.
Core c handles batch c//4 and heads [4*(c%4), 4*(c%4)+4). Each core
computes Q/K/V projections for its head slice over the full sequence,
causal flash-style attention (transposed scores, ones-column softmax
denominator), and a partial output projection against its row-slice of
W_o. The 4 partial outputs per batch are summed on the host (the
all-reduce of row-parallel W_o), which also adds b_o.

Emission is software-pipelined: projection / output-projection work is
interleaved into the attention blocks as PE filler so the scalar-engine
exp stream (the binding resource) never stalls the tensor engine.
Weights / biases / masks are loaded outside the repeat loop (resident
across iterations); only x in and partial-out per iteration.
"""
import sys

sys.path.insert(0, '/opt/trn_rl_repo')

import numpy as np
import ml_dtypes

B, S, D, H, DK = 2, 2048, 1024, 16, 64
NCORES = 8
HL = 4            # heads per core
DL = HL * DK      # head-dim slice per core (256)
NQB = S // 512    # 512-wide query blocks
NKST = S // 128   # 128-wide key tiles

_cache = {}


def _build(repeat=1, dynamic=False, stage=4):
    """stage: 1=DMAs only, 2=+QKV projections, 3=+attention, 4=full."""
    import concourse.bacc as bacc
    import concourse.mybir as mybir
    import concourse.tile as tile
    from contextlib import ExitStack, nullcontext

    f32, bf16 = mybir.dt.float32, mybir.dt.bfloat16
    Exp = mybir.ActivationFunctionType.Exp
    ET = mybir.EngineType

    nc = bacc.Bacc("TRN2", target_bir_lowering=False, debug=False, num_devices=NCORES)
    xt_d = nc.dram_tensor("xt", (D, S), bf16, kind="ExternalInput").ap()
    wq_d = nc.dram_tensor("wq", (D, DL), bf16, kind="ExternalInput").ap()
    wk_d = nc.dram_tensor("wk", (D, DL), bf16, kind="ExternalInput").ap()
    wv_d = nc.dram_tensor("wv", (D, DL), bf16, kind="ExternalInput").ap()
    wo_d = nc.dram_tensor("wo", (DL, D), bf16, kind="ExternalInput").ap()
    bqk_d = nc.dram_tensor("bqk", (DL, 2), f32, kind="ExternalInput").ap()
    bv_d = nc.dram_tensor("bv", (DL,), f32, kind="ExternalInput").ap()
    mask_d = nc.dram_tensor("masks", (4, 128, 1024), bf16, kind="ExternalInput").ap()
    po_d = nc.dram_tensor("po", (128, NQB, 8, 512), bf16, kind="ExternalOutput").ap()

    with tile.TileContext(nc) as tc:
        with ExitStack() as ctx:
            sb = ctx.enter_context(tc.tile_pool(name="sb", bufs=1))
            ps = ctx.enter_context(tc.tile_pool(name="ps", bufs=1, space="PSUM"))

            # ---- persistent SBUF tiles ----
            xt = sb.tile([128, 8, S], bf16, name="xt")
            wq_s = sb.tile([128, 8, DL], bf16, name="wq_s")
            wk_s = sb.tile([128, 8, DL], bf16, name="wk_s")
            wv_s = sb.tile([128, 8, DL], bf16, name="wv_s")
            wo_s = sb.tile([128, 2, D], bf16, name="wo_s")
            qt = [sb.tile([128, S], bf16, name=f"qt{p}") for p in range(2)]
            kt = [sb.tile([128, S], bf16, name=f"kt{p}") for p in range(2)]
            ctxt = [sb.tile([128, S], bf16, name=f"ctxt{p}") for p in range(2)]
            # v_aug: [128, ks-tile, 2 pairs x (64 h0 | one | 64 h1 | spare)]
            v_aug = sb.tile([128, NKST, 264], bf16, name="v_aug")
            masks = sb.tile([128, 4, 1024], bf16, name="masks")
            bqk_t = sb.tile([128, 2, 2], f32, name="bqk_t")
            bv_sb = sb.tile([1, DL], f32, name="bv_sb")
            bvB = sb.tile([128, DL], f32, name="bvB")

            # ---- resident (outside the repeat loop): weights, biases, masks ----
            nc.scalar.dma_start(wk_s[:], wk_d.rearrange("(k p) n -> p k n", p=128))
            nc.scalar.dma_start(wq_s[:], wq_d.rearrange("(k p) n -> p k n", p=128))
            nc.scalar.dma_start(wv_s[:], wv_d.rearrange("(k p) n -> p k n", p=128))
            nc.scalar.dma_start(wo_s[:], wo_d.rearrange("(k p) n -> p k n", p=128))
            nc.gpsimd.dma_start(bqk_t[:], bqk_d.rearrange("(p2 p) j -> p p2 j", p=128))
            nc.gpsimd.dma_start(masks[:], mask_d.rearrange("t p n -> p t n"))
            nc.gpsimd.dma_start(bv_sb[:], bv_d.rearrange("(o n) -> o n", o=1))
            nc.gpsimd.partition_broadcast(bvB[:], bv_sb[:])
            # softmax-denominator ones columns of v_aug: constant across iters
            v_ones = v_aug.rearrange("p t (pr e q) -> p t pr e q", pr=2, e=2, q=66)
            nc.gpsimd.memset(v_ones[:, :, :, :, 64:65], 1.0)

            # ---------------- per-iteration emitters ----------------
            # Projection units are emitted in two 4-matmul halves so that one
            # filler slot inside the attention pipeline stays comparable to
            # one exp period (~1us) -- a full 8-matmul unit in a single slot
            # starves the scalar engine's exp stream locally.
            def emit_qk(dst, w_s, bcol, p, blk, st, half):
                if half == 0:
                    st['pp'] = ps.tile([128, 512], f32, tag="pp", bufs=2, name="pp")
                pp = st['pp']
                for k in range(4 * half, 4 * half + 4):
                    nc.tensor.matmul(pp[:], w_s[:, k, p * 128:(p + 1) * 128],
                                     xt[:, k, blk * 512:(blk + 1) * 512],
                                     start=(k == 0), stop=(k == 7))
                if half == 1:
                    nc.vector.tensor_scalar_add(dst[p][:, blk * 512:(blk + 1) * 512], pp[:],
                                                bqk_t[:, p, bcol:bcol + 1])

            def emit_v(sp, st, half):
                if half == 0:
                    st['pp'] = ps.tile([128, 512], f32, tag="pp", bufs=2, name="pv")
                pv = st['pp']
                for k in range(4 * half, 4 * half + 4):
                    nc.tensor.matmul(pv[:, 0:DL], xt[:, k, sp * 128:(sp + 1) * 128],
                                     wv_s[:, k, :], start=(k == 0), stop=(k == 7))
                if half == 1:
                    vdst = v_aug[:, sp, :].rearrange("p (pr e q) -> p pr e q", pr=2, e=2, q=66)
                    nc.vector.tensor_add(vdst[:, :, :, 0:64],
                                         pv[:, 0:DL].rearrange("p (pr e q) -> p pr e q", pr=2, e=2, q=64),
                                         bvB[:].rearrange("p (pr e q) -> p pr e q", pr=2, e=2, q=64))

            def emit_op(qb, ot, po_sb):
                po_p = ps.tile([128, 512], f32, tag="pp", bufs=2)
                for k in range(2):
                    nc.tensor.matmul(po_p[:], wo_s[:, k, ot * 128:(ot + 1) * 128],
                                     ctxt[k][:, qb * 512:(qb + 1) * 512],
                                     start=(k == 0), stop=(k == 1))
                if qb == 3:
                    # tail block: ACT is idle there, DVE is not
                    nc.scalar.copy(po_sb[:, ot, :], po_p[:])
                else:
                    nc.vector.tensor_copy(po_sb[:, ot, :], po_p[:])

            def emit_attn(p, qb, fillers):
                """Software-pipelined scores→exp→mask→AV for one (p, qb)."""
                n_kst = 4 * qb + 4
                av = ps.tile([65, 1024], f32, tag="av", bufs=1)
                es = {}

                def emit_sc(kst):
                    # columns [0, off) of this diagonal tile are fully masked:
                    # skip them in the score matmuls, the mask-mul, and the AV
                    # matmuls (their e values are never read).
                    mi = kst - 4 * qb
                    off = max(mi, 0) * 128
                    sc = ps.tile([128, 1024], f32, tag="sc", bufs=2)
                    nc.tensor.matmul(sc[:, off:512],
                                     kt[p][0:64, kst * 128:(kst + 1) * 128],
                                     qt[p][0:64, qb * 512 + off:(qb + 1) * 512],
                                     start=True, stop=True, tile_position=(0, 0))
                    nc.tensor.matmul(sc[:, 512 + off:1024],
                                     kt[p][64:128, kst * 128:(kst + 1) * 128],
                                     qt[p][64:128, qb * 512 + off:(qb + 1) * 512],
                                     start=True, stop=True, tile_position=(64, 0))
                    e = sb.tile([128, 1024], bf16, tag="ex", bufs=6)
                    if off:
                        nc.scalar.activation(e[:, off:512], sc[:, off:512], Exp, scale=0.125)
                        nc.scalar.activation(e[:, 512 + off:1024], sc[:, 512 + off:1024],
                                             Exp, scale=0.125)
                    else:
                        nc.scalar.activation(e[:], sc[:], Exp, scale=0.125)
                    if mi >= 0:
                        nc.vector.tensor_mul(e[:, off:512], e[:, off:512],
                                             masks[:, mi, off:512])
                        nc.vector.tensor_mul(e[:, 512 + off:1024], e[:, 512 + off:1024],
                                             masks[:, mi, 512 + off:1024])
                    es[kst] = (e, off)

                def emit_av(kst):
                    e, off = es.pop(kst)
                    st, sp_ = (kst == 0), (kst == n_kst - 1)
                    nc.tensor.matmul(av[:, off:512], v_aug[:, kst, p * 132:p * 132 + 65],
                                     e[:, off:512], start=st, stop=sp_)
                    nc.tensor.matmul(av[:, 512 + off:1024], v_aug[:, kst, p * 132 + 66:p * 132 + 131],
                                     e[:, 512 + off:1024], start=st, stop=sp_)

                # lag-2 software pipeline: AV for group g runs two score-groups
                # behind, so the exp->mask chain never stalls the PE.
                emit_sc(0)
                emit_sc(1)
                for kst in range(2, n_kst):
                    emit_sc(kst)
                    if fillers:
                        fillers.pop(0)()
                    emit_av(kst - 2)
                while fillers:
                    fillers.pop(0)()
                emit_av(n_kst - 2)
                emit_av(n_kst - 1)

                # normalize: ctx /= denominator (row 64 of av)
                rc = sb.tile([1, 1024], f32, tag="rc", bufs=2)
                rb = sb.tile([64, 1024], f32, tag="rb", bufs=2)
                nc.vector.reciprocal(rc[:], av[64:65, :])
                nc.gpsimd.partition_broadcast(rb[:], rc[:])
                nc.vector.tensor_mul(ctxt[p][0:64, qb * 512:(qb + 1) * 512],
                                     av[0:64, 0:512], rb[:, 0:512])
                nc.vector.tensor_mul(ctxt[p][64:128, qb * 512:(qb + 1) * 512],
                                     av[0:64, 512:1024], rb[:, 512:1024])

            rep_ctx = (tc.For_i(0, repeat, 1, hint_engines=(ET.PE,),
                                staggered_reset=True)
                       if dynamic else nullcontext(range(repeat)))
            with rep_ctx as _it:
              for _rep in ([0] if dynamic else _it):
                # ---- per-iteration input DMA: x (transposed), seq-chunked on the
                # 2 HWDGE queues so block-0 projections start after ~1/4 of it ----
                xt_r = xt_d.rearrange("(k p) s -> p k s", p=128)
                for c_ in range(4):
                    eng = nc.sync if c_ % 2 == 0 else nc.scalar
                    eng.dma_start(xt[:, :, c_ * 512:(c_ + 1) * 512],
                                  xt_r[:, :, c_ * 512:(c_ + 1) * 512])

                if stage == 1:
                    continue

                def K_(p, b):
                    st = {}
                    return [lambda h=h: emit_qk(kt, wk_s, 1, p, b, st, h) for h in range(2)]

                def Q_(p, b):
                    st = {}
                    return [lambda h=h: emit_qk(qt, wq_s, 0, p, b, st, h) for h in range(2)]

                def V_(sp):
                    st = {}
                    return [lambda h=h: emit_v(sp, st, h) for h in range(2)]

                if stage == 2:
                    for b_ in range(4):
                        for f in K_(0, b_) + K_(1, b_):
                            f()
                    for sp in range(16):
                        for f in V_(sp):
                            f()
                    for b_ in range(4):
                        for f in Q_(0, b_) + Q_(1, b_):
                            f()
                    continue

                po_sbs = {}

                def OP_(qb, ot):
                    def f():
                        if qb not in po_sbs:
                            po_sbs[qb] = sb.tile([128, 8, 512], bf16, tag="po_s",
                                                 bufs=2, name=f"po_sb{qb % 2}")
                        emit_op(qb, ot, po_sbs[qb])
                        if ot == 7:
                            eng = nc.sync if qb % 2 == 0 else nc.scalar
                            eng.dma_start(po_d[:, qb, :, :], po_sbs.pop(qb)[:])
                    return f

                do_op = stage >= 4
                OPs = (lambda qb: [OP_(qb, ot) for ot in range(8)]) if do_op else (lambda qb: [])

                # ---- pipelined schedule: attention blocks with proj/outproj filler.
                # Stage boundaries (staggered reset): all xt readers (proj units)
                # finish by stage 1, so the next iteration's stage-0 xt DMA —
                # which may overlap our stage 3 — never races them. ----
                head = K_(0, 0) + V_(0) + V_(1) + V_(2) + V_(3) + Q_(0, 0)
                for f in head:
                    f()
                emit_attn(0, 0, K_(1, 0) + Q_(1, 0) + K_(0, 1) + V_(4))
                emit_attn(1, 0, Q_(0, 1) + K_(1, 1) + V_(5) + V_(6))
                emit_attn(0, 1, V_(7) + Q_(1, 1) + K_(0, 2) + K_(1, 2) + OPs(0)[:4])
                if dynamic:
                    tc.stage_boundary()
                emit_attn(1, 1, OPs(0)[4:] + Q_(0, 2) + V_(8) + V_(9) + V_(10) + V_(11))
                emit_attn(0, 2, Q_(1, 2) + K_(0, 3) + K_(1, 3) + OPs(1))
                emit_attn(1, 2, Q_(0, 3) + Q_(1, 3) + V_(12) + V_(13) + V_(14) + V_(15))
                if dynamic:
                    tc.stage_boundary()
                emit_attn(0, 3, OPs(2))
                if dynamic:
                    tc.stage_boundary()
                emit_attn(1, 3, [])
                for f in OPs(3):
                    f()

    nc.compile()
    return nc


def _causal_mask_ok(mask):
    m = np.asarray(mask)
    if m.shape != (S, S):
        return False
    return np.array_equal(m.astype(bool), np.triu(np.ones((S, S), bool), k=1))


def _numpy_fallback(x, mask, Wq, bq, Wk, bk, Wv, bv, Wo, bo):
    x = np.asarray(x, np.float64)
    q = (x @ Wq + bq).reshape(B, S, H, DK).transpose(0, 2, 1, 3)
    k = (x @ Wk + bk).reshape(B, S, H, DK).transpose(0, 2, 1, 3)
    v = (x @ Wv + bv).reshape(B, S, H, DK).transpose(0, 2, 1, 3)
    s = np.einsum("bhqd,bhkd->bhqk", q, k) / np.sqrt(DK)
    s = np.where(np.asarray(mask, bool), -np.inf, s)
    s = s - s.max(-1, keepdims=True)
    e = np.exp(s)
    a = e / e.sum(-1, keepdims=True)
    ctx = np.einsum("bhqk,bhkd->bhqd", a, v).transpose(0, 2, 1, 3).reshape(B, S, D)
    return (ctx @ Wo + bo).astype(np.float32)


def _tri_masks():
    m = np.zeros((4, 128, 512), np.float32)
    n = np.arange(512)
    for t in range(4):
        for p_ in range(128):
            m[t, p_, :] = (n >= t * 128 + p_)
    m = np.concatenate([m, m], axis=2)  # duplicated for the two heads per pair
    return m.astype(ml_dtypes.bfloat16)


def _make_in_maps(x, Wq, bq, Wk, bk, Wv, bv, Wo):
    Wq, Wk, Wv, Wo = (np.asarray(w, np.float32) for w in (Wq, Wk, Wv, Wo))
    bq, bk, bv = (np.asarray(b_, np.float32) for b_ in (bq, bk, bv))
    masks_np = _tri_masks()
    xts = [np.ascontiguousarray(x[b_].T.astype(ml_dtypes.bfloat16)) for b_ in range(B)]

    in_maps = []
    for c in range(NCORES):
        b_, hs = c // 4, (c % 4) * DL
        in_maps.append({
            "xt": xts[b_],
            "wq": np.ascontiguousarray(Wq[:, hs:hs + DL].astype(ml_dtypes.bfloat16)),
            "wk": np.ascontiguousarray(Wk[:, hs:hs + DL].astype(ml_dtypes.bfloat16)),
            "wv": np.ascontiguousarray(Wv[:, hs:hs + DL].astype(ml_dtypes.bfloat16)),
            "wo": np.ascontiguousarray(Wo[hs:hs + DL, :].astype(ml_dtypes.bfloat16)),
            "bqk": np.ascontiguousarray(np.stack([bq[hs:hs + DL], bk[hs:hs + DL]], 1)),
            "bv": np.ascontiguousarray(bv[hs:hs + DL]),
            "masks": masks_np,
        })
    return in_maps


def kernel(x, mask, Wq, bq, Wk, bk, Wv, bv, Wo, bo):
    x = np.ascontiguousarray(np.asarray(x, np.float32))
    if not _causal_mask_ok(mask):
        return _numpy_fallback(x, mask, Wq, bq, Wk, bk, Wv, bv, Wo, bo)

    from concourse import bass_utils

    if "nc" not in _cache:
        _cache["nc"] = _build(repeat=1)
    nc = _cache["nc"]

    bo = np.asarray(bo, np.float32)
    in_maps = _make_in_maps(x, Wq, bq, Wk, bk, Wv, bv, Wo)

    res = bass_utils.run_bass_kernel_spmd(nc, in_maps, core_ids=list(range(NCORES)))

    out = np.empty((B, S, D), np.float32)
    for b_ in range(B):
        acc = res.results[b_ * 4]["po"].astype(np.float32)
        for g in range(1, 4):
            acc = acc + res.results[b_ * 4 + g]["po"]
        # acc[p, qb, k, s] = outT[k*128+p, qb*512+s]
        out[b_] = acc.transpose(1, 3, 2, 0).reshape(S, D) + bo
    return out


# revision 25
# speedup vs baseline: 1.2380x; 1.2380x over previous
"""Multi-head causal attention on 8 Trainium2 NeuronCores.

Sharding: tensor-parallel over heads x data-parallel over batch.
Core c handles batch c//4 and heads [4*(c%4), 4*(c%4)+4). Each core
computes Q/K/V projections for its head slice over the full sequence,
causal flash-style attention (transposed scores, ones-column softmax
denominator), and a partial output projection against its row-slice of
W_o. The 4 partial outputs per batch are summed on the host (the
all-reduce of row-parallel W_o), which also adds b_o.

Emission is software-pipelined: projection / output-projection work is
interleaved into the attention blocks as PE filler so the scalar-engine
exp stream (the binding resource) never stalls the tensor engine.
Weights / biases / masks are loaded outside the repeat loop (resident
across iterations); only x in and partial-out per iteration.
"""
import sys

sys.path.insert(0, '/opt/trn_rl_repo')

import numpy as np
import ml_dtypes

B, S, D, H, DK = 2, 2048, 1024, 16, 64
NCORES = 8
HL = 4            # heads per core
DL = HL * DK      # head-dim slice per core (256)
NQB = S // 512    # 512-wide query blocks
NKST = S // 128   # 128-wide key tiles

_cache = {}


def _build(repeat=1, dynamic=False, stage=4):
    """stage: 1=DMAs only, 2=+QKV projections, 3=+attention, 4=full."""
    import concourse.bacc as bacc
    import concourse.mybir as mybir
    import concourse.tile as tile
    from contextlib import ExitStack, nullcontext

    f32, bf16 = mybir.dt.float32, mybir.dt.bfloat16
    Exp = mybir.ActivationFunctionType.Exp
    ET = mybir.EngineType

    nc = bacc.Bacc("TRN2", target_bir_lowering=False, debug=False, num_devices=NCORES)
    xt_d = nc.dram_tensor("xt", (D, S), bf16, kind="ExternalInput").ap()
    wq_d = nc.dram_tensor("wq", (D, DL), bf16, kind="ExternalInput").ap()
    wk_d = nc.dram_tensor("wk", (D, DL), bf16, kind="ExternalInput").ap()
    wv_d = nc.dram_tensor("wv", (D, DL), bf16, kind="ExternalInput").ap()
    wo_d = nc.dram_tensor("wo", (DL, D), bf16, kind="ExternalInput").ap()
    bqk_d = nc.dram_tensor("bqk", (DL, 2), f32, kind="ExternalInput").ap()
    bv_d = nc.dram_tensor("bv", (DL,), f32, kind="ExternalInput").ap()
    mask_d = nc.dram_tensor("masks", (4, 128, 1024), bf16, kind="ExternalInput").ap()
    po_d = nc.dram_tensor("po", (128, NQB, 8, 512), bf16, kind="ExternalOutput").ap()

    with tile.TileContext(nc) as tc:
        with ExitStack() as ctx:
            sb = ctx.enter_context(tc.tile_pool(name="sb", bufs=1))
            ps = ctx.enter_context(tc.tile_pool(name="ps", bufs=1, space="PSUM"))

            # ---- persistent SBUF tiles ----
            xt = sb.tile([128, 8, S], bf16, name="xt")
            wq_s = sb.tile([128, 8, DL], bf16, name="wq_s")
            wk_s = sb.tile([128, 8, DL], bf16, name="wk_s")
            wv_s = sb.tile([128, 8, DL], bf16, name="wv_s")
            wo_s = sb.tile([128, 2, D], bf16, name="wo_s")
            qt = [sb.tile([128, S], bf16, name=f"qt{p}") for p in range(2)]
            kt = [sb.tile([128, S], bf16, name=f"kt{p}") for p in range(2)]
            ctxt = [sb.tile([128, S], bf16, name=f"ctxt{p}") for p in range(2)]
            # v_aug: [128, ks-tile, 2 pairs x (64 h0 | one | 64 h1 | spare)]
            v_aug = sb.tile([128, NKST, 264], bf16, name="v_aug")
            masks = sb.tile([128, 4, 1024], bf16, name="masks")
            bqk_t = sb.tile([128, 2, 2], f32, name="bqk_t")
            bv_sb = sb.tile([1, DL], f32, name="bv_sb")
            bvB = sb.tile([128, DL], f32, name="bvB")

            # ---- resident (outside the repeat loop): weights, biases, masks ----
            nc.scalar.dma_start(wk_s[:], wk_d.rearrange("(k p) n -> p k n", p=128))
            nc.scalar.dma_start(wq_s[:], wq_d.rearrange("(k p) n -> p k n", p=128))
            nc.scalar.dma_start(wv_s[:], wv_d.rearrange("(k p) n -> p k n", p=128))
            nc.scalar.dma_start(wo_s[:], wo_d.rearrange("(k p) n -> p k n", p=128))
            nc.gpsimd.dma_start(bqk_t[:], bqk_d.rearrange("(p2 p) j -> p p2 j", p=128))
            nc.gpsimd.dma_start(masks[:], mask_d.rearrange("t p n -> p t n"))
            nc.gpsimd.dma_start(bv_sb[:], bv_d.rearrange("(o n) -> o n", o=1))
            nc.gpsimd.partition_broadcast(bvB[:], bv_sb[:])
            # softmax-denominator ones columns of v_aug: constant across iters
            v_ones = v_aug.rearrange("p t (pr e q) -> p t pr e q", pr=2, e=2, q=66)
            nc.gpsimd.memset(v_ones[:, :, :, :, 64:65], 1.0)

            # ---------------- per-iteration emitters ----------------
            # Projection units are emitted in two 4-matmul halves so that one
            # filler slot inside the attention pipeline stays comparable to
            # one exp period (~1us) -- a full 8-matmul unit in a single slot
            # starves the scalar engine's exp stream locally.
            def emit_qk(dst, w_s, bcol, p, blk, st, half):
                if half == 0:
                    st['pp'] = ps.tile([128, 512], f32, tag="pp", bufs=2, name="pp")
                pp = st['pp']
                for k in range(4 * half, 4 * half + 4):
                    nc.tensor.matmul(pp[:], w_s[:, k, p * 128:(p + 1) * 128],
                                     xt[:, k, blk * 512:(blk + 1) * 512],
                                     start=(k == 0), stop=(k == 7))
                if half == 1:
                    nc.vector.tensor_scalar_add(dst[p][:, blk * 512:(blk + 1) * 512], pp[:],
                                                bqk_t[:, p, bcol:bcol + 1])

            def emit_v(sp, st, half):
                if half == 0:
                    st['pp'] = ps.tile([128, 512], f32, tag="pp", bufs=2, name="pv")
                pv = st['pp']
                for k in range(4 * half, 4 * half + 4):
                    nc.tensor.matmul(pv[:, 0:DL], xt[:, k, sp * 128:(sp + 1) * 128],
                                     wv_s[:, k, :], start=(k == 0), stop=(k == 7))
                if half == 1:
                    vdst = v_aug[:, sp, :].rearrange("p (pr e q) -> p pr e q", pr=2, e=2, q=66)
                    nc.vector.tensor_add(vdst[:, :, :, 0:64],
                                         pv[:, 0:DL].rearrange("p (pr e q) -> p pr e q", pr=2, e=2, q=64),
                                         bvB[:].rearrange("p (pr e q) -> p pr e q", pr=2, e=2, q=64))

            def emit_op(qb, ot, po_sb):
                po_p = ps.tile([128, 512], f32, tag="pp", bufs=2)
                for k in range(2):
                    nc.tensor.matmul(po_p[:], wo_s[:, k, ot * 128:(ot + 1) * 128],
                                     ctxt[k][:, qb * 512:(qb + 1) * 512],
                                     start=(k == 0), stop=(k == 1))
                nc.vector.tensor_copy(po_sb[:, ot, :], po_p[:])

            def emit_attn(p, qb, fillers):
                """Software-pipelined scores→exp→mask→AV for one (p, qb)."""
                n_kst = 4 * qb + 4
                av = ps.tile([65, 1024], f32, tag="av", bufs=1)
                es = {}

                def emit_sc(kst):
                    # columns [0, off) of this diagonal tile are fully masked:
                    # skip them in the score matmuls, the mask-mul, and the AV
                    # matmuls (their e values are never read).
                    mi = kst - 4 * qb
                    off = max(mi, 0) * 128
                    sc = ps.tile([128, 1024], f32, tag="sc", bufs=2)
                    nc.tensor.matmul(sc[:, off:512],
                                     kt[p][0:64, kst * 128:(kst + 1) * 128],
                                     qt[p][0:64, qb * 512 + off:(qb + 1) * 512],
                                     start=True, stop=True, tile_position=(0, 0))
                    nc.tensor.matmul(sc[:, 512 + off:1024],
                                     kt[p][64:128, kst * 128:(kst + 1) * 128],
                                     qt[p][64:128, qb * 512 + off:(qb + 1) * 512],
                                     start=True, stop=True, tile_position=(64, 0))
                    e = sb.tile([128, 1024], bf16, tag="ex", bufs=4)
                    if off:
                        nc.scalar.activation(e[:, off:512], sc[:, off:512], Exp, scale=0.125)
                        nc.scalar.activation(e[:, 512 + off:1024], sc[:, 512 + off:1024],
                                             Exp, scale=0.125)
                    else:
                        nc.scalar.activation(e[:], sc[:], Exp, scale=0.125)
                    if mi >= 0:
                        nc.vector.tensor_mul(e[:, off:512], e[:, off:512],
                                             masks[:, mi, off:512])
                        nc.vector.tensor_mul(e[:, 512 + off:1024], e[:, 512 + off:1024],
                                             masks[:, mi, 512 + off:1024])
                    es[kst] = (e, off)

                def emit_av(kst):
                    e, off = es.pop(kst)
                    st, sp_ = (kst == 0), (kst == n_kst - 1)
                    nc.tensor.matmul(av[:, off:512], v_aug[:, kst, p * 132:p * 132 + 65],
                                     e[:, off:512], start=st, stop=sp_)
                    nc.tensor.matmul(av[:, 512 + off:1024], v_aug[:, kst, p * 132 + 66:p * 132 + 131],
                                     e[:, 512 + off:1024], start=st, stop=sp_)

                # lag-2 software pipeline: AV for group g runs two score-groups
                # behind, so the exp->mask chain never stalls the PE.
                emit_sc(0)
                emit_sc(1)
                for kst in range(2, n_kst):
                    emit_sc(kst)
                    if fillers:
                        fillers.pop(0)()
                    emit_av(kst - 2)
                while fillers:
                    fillers.pop(0)()
                emit_av(n_kst - 2)
                emit_av(n_kst - 1)

                # normalize: ctx /= denominator (row 64 of av)
                rc = sb.tile([1, 1024], f32, tag="rc", bufs=2)
                rb = sb.tile([64, 1024], f32, tag="rb", bufs=2)
                nc.vector.reciprocal(rc[:], av[64:65, :])
                nc.gpsimd.partition_broadcast(rb[:], rc[:])
                nc.vector.tensor_mul(ctxt[p][0:64, qb * 512:(qb + 1) * 512],
                                     av[0:64, 0:512], rb[:, 0:512])
                nc.vector.tensor_mul(ctxt[p][64:128, qb * 512:(qb + 1) * 512],
                                     av[0:64, 512:1024], rb[:, 512:1024])

            rep_ctx = (tc.For_i(0, repeat, 1, hint_engines=(ET.PE,),
                                staggered_reset=True)
                       if dynamic else nullcontext(range(repeat)))
            with rep_ctx as _it:
              for _rep in ([0] if dynamic else _it):
                # ---- per-iteration input DMA: x (transposed), seq-chunked on the
                # 2 HWDGE queues so block-0 projections start after ~1/4 of it ----
                xt_r = xt_d.rearrange("(k p) s -> p k s", p=128)
                for c_ in range(4):
                    eng = nc.sync if c_ % 2 == 0 else nc.scalar
                    eng.dma_start(xt[:, :, c_ * 512:(c_ + 1) * 512],
                                  xt_r[:, :, c_ * 512:(c_ + 1) * 512])

                if stage == 1:
                    continue

                def K_(p, b):
                    st = {}
                    return [lambda h=h: emit_qk(kt, wk_s, 1, p, b, st, h) for h in range(2)]

                def Q_(p, b):
                    st = {}
                    return [lambda h=h: emit_qk(qt, wq_s, 0, p, b, st, h) for h in range(2)]

                def V_(sp):
                    st = {}
                    return [lambda h=h: emit_v(sp, st, h) for h in range(2)]

                if stage == 2:
                    for b_ in range(4):
                        for f in K_(0, b_) + K_(1, b_):
                            f()
                    for sp in range(16):
                        for f in V_(sp):
                            f()
                    for b_ in range(4):
                        for f in Q_(0, b_) + Q_(1, b_):
                            f()
                    continue

                po_sbs = {}

                def OP_(qb, ot):
                    def f():
                        if qb not in po_sbs:
                            po_sbs[qb] = sb.tile([128, 8, 512], bf16, tag="po_s",
                                                 bufs=2, name=f"po_sb{qb % 2}")
                        emit_op(qb, ot, po_sbs[qb])
                        if ot == 7:
                            eng = nc.sync if qb % 2 == 0 else nc.scalar
                            eng.dma_start(po_d[:, qb, :, :], po_sbs.pop(qb)[:])
                    return f

                do_op = stage >= 4
                OPs = (lambda qb: [OP_(qb, ot) for ot in range(8)]) if do_op else (lambda qb: [])

                # ---- pipelined schedule: attention blocks with proj/outproj filler.
                # Stage boundaries (staggered reset): all xt readers (proj units)
                # finish by stage 1, so the next iteration's stage-0 xt DMA —
                # which may overlap our stage 3 — never races them. ----
                head = K_(0, 0) + V_(0) + V_(1) + V_(2) + V_(3) + Q_(0, 0)
                for f in head:
                    f()
                emit_attn(0, 0, K_(1, 0) + Q_(1, 0) + K_(0, 1) + V_(4))
                emit_attn(1, 0, Q_(0, 1) + K_(1, 1) + V_(5) + V_(6))
                emit_attn(0, 1, V_(7) + Q_(1, 1) + K_(0, 2) + K_(1, 2) + OPs(0)[:4])
                if dynamic:
                    tc.stage_boundary()
                emit_attn(1, 1, OPs(0)[4:] + Q_(0, 2) + V_(8) + V_(9) + V_(10) + V_(11))
                emit_attn(0, 2, Q_(1, 2) + K_(0, 3) + K_(1, 3) + OPs(1))
                emit_attn(1, 2, Q_(0, 3) + Q_(1, 3) + V_(12) + V_(13) + V_(14) + V_(15))
                if dynamic:
                    tc.stage_boundary()
                emit_attn(0, 3, OPs(2))
                if dynamic:
                    tc.stage_boundary()
                emit_attn(1, 3, [])
                for f in OPs(3):
                    f()

    nc.compile()
    return nc


def _causal_mask_ok(mask):
    m = np.asarray(mask)
    if m.shape != (S, S):
        return False
    return np.array_equal(m.astype(bool), np.triu(np.ones((S, S), bool), k=1))


def _numpy_fallback(x, mask, Wq, bq, Wk, bk, Wv, bv, Wo, bo):
    x = np.asarray(x, np.float64)
    q = (x @ Wq + bq).reshape(B, S, H, DK).transpose(0, 2, 1, 3)
    k = (x @ Wk + bk).reshape(B, S, H, DK).transpose(0, 2, 1, 3)
    v = (x @ Wv + bv).reshape(B, S, H, DK).transpose(0, 2, 1, 3)
    s = np.einsum("bhqd,bhkd->bhqk", q, k) / np.sqrt(DK)
    s = np.where(np.asarray(mask, bool), -np.inf, s)
    s = s - s.max(-1, keepdims=True)
    e = np.exp(s)
    a = e / e.sum(-1, keepdims=True)
    ctx = np.einsum("bhqk,bhkd->bhqd", a, v).transpose(0, 2, 1, 3).reshape(B, S, D)
    return (ctx @ Wo + bo).astype(np.float32)


def _tri_masks():
    m = np.zeros((4, 128, 512), np.float32)
    n = np.arange(512)
    for t in range(4):
        for p_ in range(128):
            m[t, p_, :] = (n >= t * 128 + p_)
    m = np.concatenate([m, m], axis=2)  # duplicated for the two heads per pair
    return m.astype(ml_dtypes.bfloat16)


def _make_in_maps(x, Wq, bq, Wk, bk, Wv, bv, Wo):
    Wq, Wk, Wv, Wo = (np.asarray(w, np.float32) for w in (Wq, Wk, Wv, Wo))
    bq, bk, bv = (np.asarray(b_, np.float32) for b_ in (bq, bk, bv))
    masks_np = _tri_masks()
    xts = [np.ascontiguousarray(x[b_].T.astype(ml_dtypes.bfloat16)) for b_ in range(B)]

    in_maps = []
    for c in range(NCORES):
        b_, hs = c // 4, (c % 4) * DL
        in_maps.append({
            "xt": xts[b_],
            "wq": np.ascontiguousarray(Wq[:, hs:hs + DL].astype(ml_dtypes.bfloat16)),
            "wk": np.ascontiguousarray(Wk[:, hs:hs + DL].astype(ml_dtypes.bfloat16)),
            "wv": np.ascontiguousarray(Wv[:, hs:hs + DL].astype(ml_dtypes.bfloat16)),
            "wo": np.ascontiguousarray(Wo[hs:hs + DL, :].astype(ml_dtypes.bfloat16)),
            "bqk": np.ascontiguousarray(np.stack([bq[hs:hs + DL], bk[hs:hs + DL]], 1)),
            "bv": np.ascontiguousarray(bv[hs:hs + DL]),
            "masks": masks_np,
        })
    return in_maps


def kernel(x, mask, Wq, bq, Wk, bk, Wv, bv, Wo, bo):
    x = np.ascontiguousarray(np.asarray(x, np.float32))
    if not _causal_mask_ok(mask):
        return _numpy_fallback(x, mask, Wq, bq, Wk, bk, Wv, bv, Wo, bo)

    from concourse import bass_utils

    if "nc" not in _cache:
        _cache["nc"] = _build(repeat=1)
    nc = _cache["nc"]

    bo = np.asarray(bo, np.float32)
    in_maps = _make_in_maps(x, Wq, bq, Wk, bk, Wv, bv, Wo)

    res = bass_utils.run_bass_kernel_spmd(nc, in_maps, core_ids=list(range(NCORES)))

    out = np.empty((B, S, D), np.float32)
    for b_ in range(B):
        acc = res.results[b_ * 4]["po"].astype(np.float32)
        for g in range(1, 4):
            acc = acc + res.results[b_ * 4 + g]["po"]
        # acc[p, qb, k, s] = outT[k*128+p, qb*512+s]
        out[b_] = acc.transpose(1, 3, 2, 0).reshape(S, D) + bo
    return out


# revision 28
# speedup vs baseline: 1.2554x; 1.0140x over previous
"""Multi-head causal attention on 8 Trainium2 NeuronCores.

Sharding: tensor-parallel over heads x data-parallel over batch.
Core c handles batch c//4 and heads [4*(c%4), 4*(c%4)+4). Each core
computes Q/K/V projections for its head slice over the full sequence,
causal flash-style attention (transposed scores, ones-column softmax
denominator), and a partial output projection against its row-slice of
W_o. The 4 partial outputs per batch are summed on the host (the
all-reduce of row-parallel W_o), which also adds b_o.

Emission is software-pipelined: projection / output-projection work is
interleaved into the attention blocks as PE filler so the scalar-engine
exp stream (the binding resource) never stalls the tensor engine.
Weights / biases / masks are loaded outside the repeat loop (resident
across iterations); only x in and partial-out per iteration.
"""
import sys

sys.path.insert(0, '/opt/trn_rl_repo')

import numpy as np
import ml_dtypes

B, S, D, H, DK = 2, 2048, 1024, 16, 64
NCORES = 8
HL = 4            # heads per core
DL = HL * DK      # head-dim slice per core (256)
NQB = S // 512    # 512-wide query blocks
NKST = S // 128   # 128-wide key tiles

_cache = {}


def _build(repeat=1, dynamic=False, stage=4):
    """stage: 1=DMAs only, 2=+QKV projections, 3=+attention, 4=full."""
    import concourse.bacc as bacc
    import concourse.mybir as mybir
    import concourse.tile as tile
    from contextlib import ExitStack, nullcontext

    f32, bf16 = mybir.dt.float32, mybir.dt.bfloat16
    Exp = mybir.ActivationFunctionType.Exp
    ET = mybir.EngineType

    nc = bacc.Bacc("TRN2", target_bir_lowering=False, debug=False, num_devices=NCORES)
    xt_d = nc.dram_tensor("xt", (D, S), bf16, kind="ExternalInput").ap()
    wq_d = nc.dram_tensor("wq", (D, DL), bf16, kind="ExternalInput").ap()
    wk_d = nc.dram_tensor("wk", (D, DL), bf16, kind="ExternalInput").ap()
    wv_d = nc.dram_tensor("wv", (D, DL), bf16, kind="ExternalInput").ap()
    wo_d = nc.dram_tensor("wo", (DL, D), bf16, kind="ExternalInput").ap()
    bqk_d = nc.dram_tensor("bqk", (DL, 2), f32, kind="ExternalInput").ap()
    bv_d = nc.dram_tensor("bv", (DL,), f32, kind="ExternalInput").ap()
    mask_d = nc.dram_tensor("masks", (4, 128, 1024), bf16, kind="ExternalInput").ap()
    po_d = nc.dram_tensor("po", (128, NQB, 8, 512), bf16, kind="ExternalOutput").ap()

    with tile.TileContext(nc) as tc:
        with ExitStack() as ctx:
            sb = ctx.enter_context(tc.tile_pool(name="sb", bufs=1))
            ps = ctx.enter_context(tc.tile_pool(name="ps", bufs=1, space="PSUM"))

            # ---- persistent SBUF tiles ----
            xt = sb.tile([128, 8, S], bf16, name="xt")
            wq_s = sb.tile([128, 8, DL], bf16, name="wq_s")
            wk_s = sb.tile([128, 8, DL], bf16, name="wk_s")
            wv_s = sb.tile([128, 8, DL], bf16, name="wv_s")
            wo_s = sb.tile([128, 2, D], bf16, name="wo_s")
            qt = [sb.tile([128, S], bf16, name=f"qt{p}") for p in range(2)]
            kt = [sb.tile([128, S], bf16, name=f"kt{p}") for p in range(2)]
            ctxt = [sb.tile([128, S], bf16, name=f"ctxt{p}") for p in range(2)]
            # v_aug: [128, ks-tile, 2 pairs x (64 h0 | one | 64 h1 | spare)]
            v_aug = sb.tile([128, NKST, 264], bf16, name="v_aug")
            masks = sb.tile([128, 4, 1024], bf16, name="masks")
            bqk_t = sb.tile([128, 2, 2], f32, name="bqk_t")
            bv_sb = sb.tile([1, DL], f32, name="bv_sb")
            bvB = sb.tile([128, DL], f32, name="bvB")

            # ---- resident (outside the repeat loop): weights, biases, masks ----
            nc.scalar.dma_start(wk_s[:], wk_d.rearrange("(k p) n -> p k n", p=128))
            nc.scalar.dma_start(wq_s[:], wq_d.rearrange("(k p) n -> p k n", p=128))
            nc.scalar.dma_start(wv_s[:], wv_d.rearrange("(k p) n -> p k n", p=128))
            nc.scalar.dma_start(wo_s[:], wo_d.rearrange("(k p) n -> p k n", p=128))
            nc.gpsimd.dma_start(bqk_t[:], bqk_d.rearrange("(p2 p) j -> p p2 j", p=128))
            nc.gpsimd.dma_start(masks[:], mask_d.rearrange("t p n -> p t n"))
            nc.gpsimd.dma_start(bv_sb[:], bv_d.rearrange("(o n) -> o n", o=1))
            nc.gpsimd.partition_broadcast(bvB[:], bv_sb[:])
            # softmax-denominator ones columns of v_aug: constant across iters
            v_ones = v_aug.rearrange("p t (pr e q) -> p t pr e q", pr=2, e=2, q=66)
            nc.gpsimd.memset(v_ones[:, :, :, :, 64:65], 1.0)

            # ---------------- per-iteration emitters ----------------
            # Projection units are emitted in two 4-matmul halves so that one
            # filler slot inside the attention pipeline stays comparable to
            # one exp period (~1us) -- a full 8-matmul unit in a single slot
            # starves the scalar engine's exp stream locally.
            def emit_qk(dst, w_s, bcol, p, blk, st, half):
                if half == 0:
                    st['pp'] = ps.tile([128, 512], f32, tag="pp", bufs=2, name="pp")
                pp = st['pp']
                for k in range(4 * half, 4 * half + 4):
                    nc.tensor.matmul(pp[:], w_s[:, k, p * 128:(p + 1) * 128],
                                     xt[:, k, blk * 512:(blk + 1) * 512],
                                     start=(k == 0), stop=(k == 7))
                if half == 1:
                    nc.vector.tensor_scalar_add(dst[p][:, blk * 512:(blk + 1) * 512], pp[:],
                                                bqk_t[:, p, bcol:bcol + 1])

            def emit_v(sp, st, half):
                if half == 0:
                    st['pp'] = ps.tile([128, 512], f32, tag="pp", bufs=2, name="pv")
                pv = st['pp']
                for k in range(4 * half, 4 * half + 4):
                    nc.tensor.matmul(pv[:, 0:DL], xt[:, k, sp * 128:(sp + 1) * 128],
                                     wv_s[:, k, :], start=(k == 0), stop=(k == 7))
                if half == 1:
                    vdst = v_aug[:, sp, :].rearrange("p (pr e q) -> p pr e q", pr=2, e=2, q=66)
                    nc.vector.tensor_add(vdst[:, :, :, 0:64],
                                         pv[:, 0:DL].rearrange("p (pr e q) -> p pr e q", pr=2, e=2, q=64),
                                         bvB[:].rearrange("p (pr e q) -> p pr e q", pr=2, e=2, q=64))

            def emit_op(qb, ot, po_sb):
                po_p = ps.tile([128, 512], f32, tag="pp", bufs=2)
                for k in range(2):
                    nc.tensor.matmul(po_p[:], wo_s[:, k, ot * 128:(ot + 1) * 128],
                                     ctxt[k][:, qb * 512:(qb + 1) * 512],
                                     start=(k == 0), stop=(k == 1))
                nc.vector.tensor_copy(po_sb[:, ot, :], po_p[:])

            def make_attn(p, qb):
                """Closures for one (p, qb) attention block: sc(g), av(g), norm().

                Driven by run_blocks so the last AV + normalize of block i are
                emitted after block i+1's first two score groups — the PE keeps
                streaming across block boundaries while the av psum bank
                (bufs=1) drains.
                """
                n_kst = 4 * qb + 4
                av = ps.tile([65, 1024], f32, tag="av", bufs=1)
                es = {}

                def emit_sc(kst):
                    # columns [0, off) of this diagonal tile are fully masked:
                    # skip them in the score matmuls, the mask-mul, and the AV
                    # matmuls (their e values are never read).
                    mi = kst - 4 * qb
                    off = max(mi, 0) * 128
                    sc = ps.tile([128, 1024], f32, tag="sc", bufs=2)
                    nc.tensor.matmul(sc[:, off:512],
                                     kt[p][0:64, kst * 128:(kst + 1) * 128],
                                     qt[p][0:64, qb * 512 + off:(qb + 1) * 512],
                                     start=True, stop=True, tile_position=(0, 0))
                    nc.tensor.matmul(sc[:, 512 + off:1024],
                                     kt[p][64:128, kst * 128:(kst + 1) * 128],
                                     qt[p][64:128, qb * 512 + off:(qb + 1) * 512],
                                     start=True, stop=True, tile_position=(64, 0))
                    e = sb.tile([128, 1024], bf16, tag="ex", bufs=4)
                    if off:
                        nc.scalar.activation(e[:, off:512], sc[:, off:512], Exp, scale=0.125)
                        nc.scalar.activation(e[:, 512 + off:1024], sc[:, 512 + off:1024],
                                             Exp, scale=0.125)
                    else:
                        nc.scalar.activation(e[:], sc[:], Exp, scale=0.125)
                    if mi >= 0:
                        nc.vector.tensor_mul(e[:, off:512], e[:, off:512],
                                             masks[:, mi, off:512])
                        nc.vector.tensor_mul(e[:, 512 + off:1024], e[:, 512 + off:1024],
                                             masks[:, mi, 512 + off:1024])
                    es[kst] = (e, off)

                def emit_av(kst):
                    e, off = es.pop(kst)
                    st, sp_ = (kst == 0), (kst == n_kst - 1)
                    nc.tensor.matmul(av[:, off:512], v_aug[:, kst, p * 132:p * 132 + 65],
                                     e[:, off:512], start=st, stop=sp_)
                    nc.tensor.matmul(av[:, 512 + off:1024], v_aug[:, kst, p * 132 + 66:p * 132 + 131],
                                     e[:, 512 + off:1024], start=st, stop=sp_)

                def emit_norm():
                    # normalize: ctx /= denominator (row 64 of av)
                    rc = sb.tile([1, 1024], f32, tag="rc", bufs=2)
                    rb = sb.tile([64, 1024], f32, tag="rb", bufs=2)
                    nc.vector.reciprocal(rc[:], av[64:65, :])
                    nc.gpsimd.partition_broadcast(rb[:], rc[:])
                    nc.vector.tensor_mul(ctxt[p][0:64, qb * 512:(qb + 1) * 512],
                                         av[0:64, 0:512], rb[:, 0:512])
                    nc.vector.tensor_mul(ctxt[p][64:128, qb * 512:(qb + 1) * 512],
                                         av[0:64, 512:1024], rb[:, 512:1024])

                return {"G": n_kst, "sc": emit_sc, "av": emit_av, "norm": emit_norm}

            def run_blocks(blocks, boundaries):
                """Emit attention blocks with lag-2 AV pipelining, PE filler
                injection, and cross-block overlap: block i's last AV +
                normalize are emitted after block i+1's first two score
                groups."""
                prev = None
                for bi, (p, qb, fillers) in enumerate(blocks):
                    b = make_attn(p, qb)
                    b["sc"](0)
                    b["sc"](1)
                    if prev is not None:
                        prev["av"](prev["G"] - 1)
                        prev["norm"]()
                    for g in range(2, b["G"]):
                        b["sc"](g)
                        if fillers:
                            fillers.pop(0)()
                        b["av"](g - 2)
                    while fillers:
                        fillers.pop(0)()
                    b["av"](b["G"] - 2)
                    prev = b
                    if bi in boundaries and dynamic:
                        tc.stage_boundary()
                prev["av"](prev["G"] - 1)
                prev["norm"]()

            rep_ctx = (tc.For_i(0, repeat, 1, hint_engines=(ET.PE,),
                                staggered_reset=True)
                       if dynamic else nullcontext(range(repeat)))
            with rep_ctx as _it:
              for _rep in ([0] if dynamic else _it):
                # ---- per-iteration input DMA: x (transposed), seq-chunked on the
                # 2 HWDGE queues so block-0 projections start after ~1/4 of it ----
                xt_r = xt_d.rearrange("(k p) s -> p k s", p=128)
                for c_ in range(4):
                    eng = nc.sync if c_ % 2 == 0 else nc.scalar
                    eng.dma_start(xt[:, :, c_ * 512:(c_ + 1) * 512],
                                  xt_r[:, :, c_ * 512:(c_ + 1) * 512])

                if stage == 1:
                    continue

                def K_(p, b):
                    st = {}
                    return [lambda h=h: emit_qk(kt, wk_s, 1, p, b, st, h) for h in range(2)]

                def Q_(p, b):
                    st = {}
                    return [lambda h=h: emit_qk(qt, wq_s, 0, p, b, st, h) for h in range(2)]

                def V_(sp):
                    st = {}
                    return [lambda h=h: emit_v(sp, st, h) for h in range(2)]

                if stage == 2:
                    for b_ in range(4):
                        for f in K_(0, b_) + K_(1, b_):
                            f()
                    for sp in range(16):
                        for f in V_(sp):
                            f()
                    for b_ in range(4):
                        for f in Q_(0, b_) + Q_(1, b_):
                            f()
                    continue

                po_sbs = {}

                def OP_(qb, ot):
                    def f():
                        if qb not in po_sbs:
                            po_sbs[qb] = sb.tile([128, 8, 512], bf16, tag="po_s",
                                                 bufs=2, name=f"po_sb{qb % 2}")
                        emit_op(qb, ot, po_sbs[qb])
                        if ot == 7:
                            eng = nc.sync if qb % 2 == 0 else nc.scalar
                            eng.dma_start(po_d[:, qb, :, :], po_sbs.pop(qb)[:])
                    return f

                do_op = stage >= 4
                OPs = (lambda qb: [OP_(qb, ot) for ot in range(8)]) if do_op else (lambda qb: [])

                # ---- pipelined schedule: attention blocks with proj/outproj filler.
                # Stage boundaries (staggered reset): all xt readers (proj units)
                # finish by stage 1, so the next iteration's stage-0 xt DMA —
                # which may overlap our stage 3 — never races them. ----
                head = K_(0, 0) + V_(0) + V_(1) + V_(2) + V_(3) + Q_(0, 0)
                for f in head:
                    f()
                op2 = OPs(2)
                blocks = [
                    (0, 0, K_(1, 0) + Q_(1, 0) + K_(0, 1) + V_(4)),
                    (1, 0, Q_(0, 1) + K_(1, 1) + V_(5) + V_(6)),
                    (0, 1, V_(7) + Q_(1, 1) + K_(0, 2) + K_(1, 2) + OPs(0)[:4]),
                    (1, 1, OPs(0)[4:] + Q_(0, 2) + V_(8) + V_(9) + V_(10) + V_(11)),
                    (0, 2, Q_(1, 2) + K_(0, 3) + K_(1, 3) + OPs(1)),
                    (1, 2, Q_(0, 3) + Q_(1, 3) + V_(12) + V_(13) + V_(14) + V_(15)),
                    (0, 3, op2[:4]),
                    (1, 3, op2[4:]),
                ]
                run_blocks(blocks, boundaries={2, 5, 6})
                for f in OPs(3):
                    f()

    nc.compile()
    return nc


def _causal_mask_ok(mask):
    m = np.asarray(mask)
    if m.shape != (S, S):
        return False
    return np.array_equal(m.astype(bool), np.triu(np.ones((S, S), bool), k=1))


def _numpy_fallback(x, mask, Wq, bq, Wk, bk, Wv, bv, Wo, bo):
    x = np.asarray(x, np.float64)
    q = (x @ Wq + bq).reshape(B, S, H, DK).transpose(0, 2, 1, 3)
    k = (x @ Wk + bk).reshape(B, S, H, DK).transpose(0, 2, 1, 3)
    v = (x @ Wv + bv).reshape(B, S, H, DK).transpose(0, 2, 1, 3)
    s = np.einsum("bhqd,bhkd->bhqk", q, k) / np.sqrt(DK)
    s = np.where(np.asarray(mask, bool), -np.inf, s)
    s = s - s.max(-1, keepdims=True)
    e = np.exp(s)
    a = e / e.sum(-1, keepdims=True)
    ctx = np.einsum("bhqk,bhkd->bhqd", a, v).transpose(0, 2, 1, 3).reshape(B, S, D)
    return (ctx @ Wo + bo).astype(np.float32)


def _tri_masks():
    m = np.zeros((4, 128, 512), np.float32)
    n = np.arange(512)
    for t in range(4):
        for p_ in range(128):
            m[t, p_, :] = (n >= t * 128 + p_)
    m = np.concatenate([m, m], axis=2)  # duplicated for the two heads per pair
    return m.astype(ml_dtypes.bfloat16)


def _make_in_maps(x, Wq, bq, Wk, bk, Wv, bv, Wo):
    Wq, Wk, Wv, Wo = (np.asarray(w, np.float32) for w in (Wq, Wk, Wv, Wo))
    bq, bk, bv = (np.asarray(b_, np.float32) for b_ in (bq, bk, bv))
    masks_np = _tri_masks()
    xts = [np.ascontiguousarray(x[b_].T.astype(ml_dtypes.bfloat16)) for b_ in range(B)]

    in_maps = []
    for c in range(NCORES):
        b_, hs = c // 4, (c % 4) * DL
        in_maps.append({
            "xt": xts[b_],
            "wq": np.ascontiguousarray(Wq[:, hs:hs + DL].astype(ml_dtypes.bfloat16)),
            "wk": np.ascontiguousarray(Wk[:, hs:hs + DL].astype(ml_dtypes.bfloat16)),
            "wv": np.ascontiguousarray(Wv[:, hs:hs + DL].astype(ml_dtypes.bfloat16)),
            "wo": np.ascontiguousarray(Wo[hs:hs + DL, :].astype(ml_dtypes.bfloat16)),
            "bqk": np.ascontiguousarray(np.stack([bq[hs:hs + DL], bk[hs:hs + DL]], 1)),
            "bv": np.ascontiguousarray(bv[hs:hs + DL]),
            "masks": masks_np,
        })
    return in_maps


def kernel(x, mask, Wq, bq, Wk, bk, Wv, bv, Wo, bo):
    x = np.ascontiguousarray(np.asarray(x, np.float32))
    if not _causal_mask_ok(mask):
        return _numpy_fallback(x, mask, Wq, bq, Wk, bk, Wv, bv, Wo, bo)

    from concourse import bass_utils

    if "nc" not in _cache:
        _cache["nc"] = _build(repeat=1)
    nc = _cache["nc"]

    bo = np.asarray(bo, np.float32)
    in_maps = _make_in_maps(x, Wq, bq, Wk, bk, Wv, bv, Wo)

    res = bass_utils.run_bass_kernel_spmd(nc, in_maps, core_ids=list(range(NCORES)))

    out = np.empty((B, S, D), np.float32)
    for b_ in range(B):
        acc = res.results[b_ * 4]["po"].astype(np.float32)
        for g in range(1, 4):
            acc = acc + res.results[b_ * 4 + g]["po"]
        # acc[p, qb, k, s] = outT[k*128+p, qb*512+s]
        out[b_] = acc.transpose(1, 3, 2, 0).reshape(S, D) + bo
    return out
